# revision 3
# baseline (speedup 1.0000x reference)
"""Trainium2 Bass kernel for nn_MultiHeadAttention_77232101917088.

Causal MHA where only the LAST token's projected output is returned:
    out = (softmax_causal(q k^T / sqrt(hd)) v)[:, -1, :] @ Wo + bo

Only the last query row survives, so the problem collapses (last causal
row attends to every position; K/V are never materialized):
    qT[m, b]      = sum_d x[b,-1,d] Wq[d, m]
    u[b,h,d]      = sum_e Wk[d, h*128+e] q[b, h*128+e]
    scores[b,h,j] = sum_d x[b,j,d] u[b,h,d]
    p             = softmax_j(scores / sqrt(hd))   (no max-subtract:
                    |scores*ISCALE| <= ~5, exp is safe in fp32)
    w[b,h,d]      = sum_j p[b,h,j] x[b,j,d]
    ctx[b, h*128:+128] = w[b,h,:] @ Wv[:, h*128:+128]
    out           = ctx @ Wo + bo

Sharding: model dim d = 2048 split into 8 chunks of 256 (one per core).
Per-core: partial qT from Wq row-chunk -> AllReduce(qT, 8 KB fp16) ->
per-head u on the local d-chunk -> partial scores -> AllReduce(scores,
256 KB fp32) -> redundant softmax everywhere -> w on local d-chunk ->
partial ctx^T -> AllReduce(ctx^T, 8 KB fp16) -> out column shard with
Wo[:, chunk].  Host concatenates shards and adds bo.  Small collectives
use Shared outputs (Mesh one-shot algorithm, ~8 us each).
"""

import numpy as np

import concourse.bacc as bacc
import concourse.bass as bass
import concourse.mybir as mybir
import concourse.tile as tile
from concourse.masks import make_identity
from concourse.bass_utils import run_bass_kernel_spmd

P = 128          # partitions
B = 2            # batch
S = 2048         # sequence length
D = 2048         # model dim
NH = 16          # heads
HD = 128         # head dim
NC = 8           # cores
CH = D // NC     # per-core model-dim chunk (256)
CT = CH // P     # chunk subtiles (2)
DT = D // P      # full-depth subtiles (16)
JT = S // P      # sequence subtiles (16)
BH = B * NH      # 32
NJC = 4          # j chunks of 512 for score matmul
JC = S // NJC    # 512
ISCALE = 1.0 / np.sqrt(HD)

FP32 = mybir.dt.float32
FP16 = mybir.dt.float16

G8 = [list(range(NC))]


def _build_program():
    nc = bacc.Bacc(
        "TRN2",
        target_bir_lowering=False,
        debug=False,
        enable_asserts=False,
        num_devices=NC,
    )

    # ---- per-core DRAM inputs --------------------------------------------
    xlastT = nc.dram_tensor("xlastT", [CH, B], FP16, kind="ExternalInput").ap()
    wq = nc.dram_tensor("wq", [CH, D], FP16, kind="ExternalInput").ap()
    wkT = nc.dram_tensor("wkT", [D, CH], FP16, kind="ExternalInput").ap()
    xT = nc.dram_tensor("xT", [B, CH, S], FP16, kind="ExternalInput").ap()
    xn = nc.dram_tensor("xn", [B, S, CH], FP16, kind="ExternalInput").ap()
    wv = nc.dram_tensor("wv", [CH, D], FP16, kind="ExternalInput").ap()
    woc = nc.dram_tensor("woc", [D, CH], FP16, kind="ExternalInput").ap()

    # out_sh[b, m] = out[b, i*CH + m]  (fully reduced column shard)
    out_sh = nc.dram_tensor("out_sh", [B, CH], FP32, kind="ExternalOutput").ap()

    # collective staging (Local in -> Shared out enables Mesh one-shot)
    arq_in = nc.dram_tensor("arq_in", [D, B], FP16).ap()
    arq_out = nc.dram_tensor("arq_out", [D, B], FP16, addr_space="Shared").ap()
    ars_in = nc.dram_tensor("ars_in", [B, NH, S], FP32).ap()
    ars_out = nc.dram_tensor("ars_out", [B, NH, S], FP32, addr_space="Shared").ap()
    arc_in = nc.dram_tensor("arc_in", [D, B], FP16).ap()
    arc_out = nc.dram_tensor("arc_out", [D, B], FP16, addr_space="Shared").ap()

    with tile.TileContext(nc) as tc:
        with (
            tc.tile_pool(name="persist", bufs=1) as pp,
            tc.tile_pool(name="work", bufs=1) as wp,
            tc.tile_pool(name="psA", bufs=3, space="PSUM") as psA,
            tc.tile_pool(name="psS", bufs=1, space="PSUM") as psS,
        ):
            # ---- loads: critical-path first ------------------------------
            xlastT_sb = pp.tile([P, CT, B], FP16, name="xlastT_sb")
            nc.sync.dma_start(xlastT_sb[:], xlastT.rearrange("(c p) b -> p c b", p=P))
            wq_sb = pp.tile([P, CT, D], FP16, name="wq_sb")
            nc.sync.dma_start(wq_sb[:], wq.rearrange("(c p) m -> p c m", p=P))

            wkT_sb = pp.tile([P, DT, CH], FP16, name="wkT_sb")
            nc.scalar.dma_start(wkT_sb[:], wkT.rearrange("(t p) c -> p t c", p=P))
            xT_sb = [pp.tile([P, CT, S], FP16, name=f"xT_sb{b}") for b in range(B)]
            for b in range(B):
                nc.scalar.dma_start(xT_sb[b][:], xT[b].rearrange("(c p) j -> p c j", p=P))

            xn_sb = [pp.tile([P, JT, CH], FP16, name=f"xn_sb{b}") for b in range(B)]
            for b in range(B):
                nc.sync.dma_start(xn_sb[b][:], xn[b].rearrange("(t p) c -> p t c", p=P))
            wv_sb = pp.tile([P, CT, D], FP16, name="wv_sb")
            nc.sync.dma_start(wv_sb[:], wv.rearrange("(c p) m -> p c m", p=P))
            woc_sb = pp.tile([P, DT, CH], FP16, name="woc_sb")
            nc.sync.dma_start(woc_sb[:], woc.rearrange("(t p) c -> p t c", p=P))

            ident_sb = pp.tile([BH, BH], FP16, name="ident_sb")
            make_identity(nc, ident_sb[:])

            # ---- A: partial qT[m, b] over local d-rows (32 matmuls) ------
            ps_qT = psA.tile([P, DT, B], FP32, name="ps_qT", tag="psA")
            for t in range(DT):
                for ds in range(CT):
                    nc.tensor.matmul(
                        ps_qT[:, t, :],
                        lhsT=wq_sb[:, ds, t * P:(t + 1) * P],
                        rhs=xlastT_sb[:, ds, :],
                        start=(ds == 0),
                        stop=(ds == CT - 1),
                    )
            qTp_sb = wp.tile([P, DT, B], FP16, name="qTp_sb")
            nc.vector.tensor_copy(qTp_sb[:], ps_qT[:])
            nc.gpsimd.dma_start(arq_in.rearrange("(t p) b -> p t b", p=P), qTp_sb[:])
            nc.gpsimd.collective_compute(
                "AllReduce", mybir.AluOpType.add, replica_groups=G8,
                ins=[arq_in.opt()], outs=[arq_out.opt()],
            )
            qT_sb = wp.tile([P, DT, B], FP16, name="qT_sb")
            nc.gpsimd.dma_start(qT_sb[:], arq_out.rearrange("(t p) b -> p t b", p=P))

            # ---- B: per-head uT[c, b] = Wk-block^T q-block (32 matmuls) --
            ps_u = psA.tile([P, CT, NH, B], FP32, name="ps_u", tag="psA")
            for h in range(NH):
                for ds in range(CT):
                    nc.tensor.matmul(
                        ps_u[:, ds, h, :],
                        lhsT=wkT_sb[:, h, ds * P:(ds + 1) * P],
                        rhs=qT_sb[:, h, :],
                        start=True, stop=True,
                    )
            uT_sb = wp.tile([P, CT, B, NH], FP16, name="uT_sb")
            for ds in range(CT):
                for b in range(B):
                    nc.vector.tensor_copy(uT_sb[:, ds, b, :], ps_u[:, ds, :, b])

            # ---- C: partial scores [nh, j] per batch ---------------------
            sc_sb = [wp.tile([NH, S], FP32, name=f"sc_sb{b}") for b in range(B)]
            for b in range(B):
                ps_s = psS.tile([NH, S], FP32, name="ps_s", tag="psS")
                for jc in range(NJC):
                    for ds in range(CT):
                        nc.tensor.matmul(
                            ps_s[:, jc * JC:(jc + 1) * JC],
                            lhsT=uT_sb[:, ds, b, :],
                            rhs=xT_sb[b][:, ds, jc * JC:(jc + 1) * JC],
                            start=(ds == 0),
                            stop=(ds == CT - 1),
                        )
                nc.vector.tensor_copy(sc_sb[b][:, :S // 2], ps_s[:, :S // 2])
                nc.scalar.copy(sc_sb[b][:, S // 2:], ps_s[:, S // 2:])
                nc.gpsimd.dma_start(ars_in[b], sc_sb[b][:])
            nc.gpsimd.collective_compute(
                "AllReduce", mybir.AluOpType.add, replica_groups=G8,
                ins=[ars_in.opt()], outs=[ars_out.opt()],
            )
            sc2_sb = wp.tile([BH, S], FP32, name="sc2_sb")
            for b in range(B):
                nc.gpsimd.dma_start(sc2_sb[b * NH:(b + 1) * NH, :], ars_out[b])

            # ---- D: softmax without max-subtract; 1/z folded into e ------
            e16_sb = wp.tile([BH, S], FP16, name="e16_sb")
            z_sb = wp.tile([BH, 1], FP32, name="z_sb")
            nc.scalar.activation(
                e16_sb[:], sc2_sb[:], mybir.ActivationFunctionType.Exp,
                scale=float(ISCALE), accum_out=z_sb[:],
            )
            rz_sb = wp.tile([BH, 1], FP32, name="rz_sb")
            nc.vector.reciprocal(rz_sb[:], z_sb[:])
            nc.vector.tensor_scalar_mul(e16_sb[:], e16_sb[:], rz_sb[:])
            eT_sb = wp.tile([P, JT, BH], FP16, name="eT_sb")
            for jt in range(JT):
                ps_t = psA.tile([P, BH], FP16, name="ps_t", tag="psA")
                nc.tensor.transpose(
                    ps_t[:], e16_sb[:, jt * P:(jt + 1) * P], ident_sb[:]
                )
                nc.vector.tensor_copy(eT_sb[:, jt, :], ps_t[:])

            # ---- E: w2[nh, c] = sum_j eT[j, nh] xn[j, c] per batch -------
            w16_sb = [wp.tile([NH, CH], FP16, name=f"w16_sb{b}") for b in range(B)]
            for b in range(B):
                ps_w = psA.tile([NH, CH], FP32, name="ps_w", tag="psA")
                for jt in range(JT):
                    nc.tensor.matmul(
                        ps_w[:],
                        lhsT=eT_sb[:, jt, b * NH:(b + 1) * NH],
                        rhs=xn_sb[b][:, jt, :],
                        start=(jt == 0),
                        stop=(jt == JT - 1),
                    )
                nc.vector.tensor_copy(w16_sb[b][:], ps_w[:])

            # ---- wT: [c, b] per-head layout for F ------------------------
            wT_sb = wp.tile([P, CT, B, NH], FP16, name="wT_sb")
            for b in range(B):
                for ds in range(CT):
                    ps_wt = psA.tile([P, NH], FP16, name="ps_wt", tag="psA")
                    nc.tensor.transpose(
                        ps_wt[:], w16_sb[b][:, ds * P:(ds + 1) * P],
                        ident_sb[:NH, :NH],
                    )
                    nc.vector.tensor_copy(wT_sb[:, ds, b, :], ps_wt[:])

            # ---- F: partial ctx^T[o, b] per head (32 matmuls) ------------
            ps_c = psA.tile([P, NH, B], FP32, name="ps_c", tag="psA")
            for h in range(NH):
                for ds in range(CT):
                    nc.tensor.matmul(
                        ps_c[:, h, :],
                        lhsT=wv_sb[:, ds, h * P:(h + 1) * P],
                        rhs=wT_sb[:, ds, :, h],
                        start=(ds == 0),
                        stop=(ds == CT - 1),
                    )
            ctxp_sb = wp.tile([P, NH, B], FP16, name="ctxp_sb")
            nc.vector.tensor_copy(ctxp_sb[:], ps_c[:])
            nc.gpsimd.dma_start(arc_in.rearrange("(h p) b -> p h b", p=P), ctxp_sb[:])
            nc.gpsimd.collective_compute(
                "AllReduce", mybir.AluOpType.add, replica_groups=G8,
                ins=[arc_in.opt()], outs=[arc_out.opt()],
            )
            ctxf_sb = wp.tile([P, DT, B], FP16, name="ctxf_sb")
            nc.gpsimd.dma_start(ctxf_sb[:], arc_out.rearrange("(t p) b -> p t b", p=P))

            # ---- G: out[:, chunk] = ctx @ Wo[:, chunk] -------------------
            ps_o = psA.tile([B, CH], FP32, name="ps_o", tag="psA")
            for t in range(DT):
                nc.tensor.matmul(
                    ps_o[:],
                    lhsT=ctxf_sb[:, t, :],
                    rhs=woc_sb[:, t, :],
                    start=(t == 0),
                    stop=(t == DT - 1),
                )
            o_sb = wp.tile([B, CH], FP32, name="o_sb")
            nc.vector.tensor_copy(o_sb[:], ps_o[:])
            nc.sync.dma_start(out_sh[:], o_sb[:])

    nc.compile()
    return nc


_PROGRAM = None


def _get_program():
    global _PROGRAM
    if _PROGRAM is None:
        _PROGRAM = _build_program()
    return _PROGRAM


def _shard_inputs(x, Wq, Wk, Wv, Wo):
    x16 = x.astype(np.float16)
    xT16 = np.ascontiguousarray(x16.transpose(0, 2, 1))       # [B, D, S]
    xlastT16 = np.ascontiguousarray(x16[:, -1, :].T)          # [D, B]
    wq16 = Wq.astype(np.float16)
    wkT16 = np.ascontiguousarray(Wk.T)                        # [D(he), D(d)]
    wv16 = Wv.astype(np.float16)
    wo16 = Wo.astype(np.float16)
    in_maps = []
    for i in range(NC):
        sl = slice(i * CH, (i + 1) * CH)
        in_maps.append({
            "xlastT": np.ascontiguousarray(xlastT16[sl, :]),
            "wq": np.ascontiguousarray(wq16[sl, :]),
            "wkT": np.ascontiguousarray(wkT16[:, sl]).astype(np.float16),
            "xT": np.ascontiguousarray(xT16[:, sl, :]),
            "xn": np.ascontiguousarray(x16[:, :, sl]),
            "wv": np.ascontiguousarray(wv16[sl, :]),
            "woc": np.ascontiguousarray(wo16[:, sl]),
        })
    return in_maps


def kernel(x, Wq, Wk, Wv, Wo, bo, _trace=False, _trace_cores=None):
    x = np.asarray(x, dtype=np.float32)
    Wq = np.asarray(Wq, dtype=np.float32)
    Wk = np.asarray(Wk, dtype=np.float32)
    Wv = np.asarray(Wv, dtype=np.float32)
    Wo = np.asarray(Wo, dtype=np.float32)
    bo = np.asarray(bo, dtype=np.float32)

    nc = _get_program()
    in_maps = _shard_inputs(x, Wq, Wk, Wv, Wo)
    res = run_bass_kernel_spmd(
        nc, in_maps, core_ids=list(range(NC)),
        trace=_trace, trace_cores=_trace_cores,
    )
    out = np.concatenate([res.results[i]["out_sh"] for i in range(NC)], axis=1)
    out = out + bo[None, :]
    if _trace:
        kernel._last_results = res
    return out.astype(np.float32)
